# revision 20
# baseline (speedup 1.0000x reference)
"""Trainium2 Bass kernel for nn_KnowledgeCircuit (moe_routing).

  h   = einsum('bsd,ndr,bsn->bsr', x, feature_know, feature_know_w)
  out = einsum('bsr,bsn,nrd->bsd', h, restore_know_w, restore_know)

Shapes: B=4, S=2048, D=1024, N=64, R=128.

Sharding: data-parallel over the B*S = 8192 tokens -> 1024 tokens per
NeuronCore across 8 cores; neuron pools (fk, rk) replicated. No
collectives.

Host prep: x is pre-transposed to [D, T] and packed to bf16; fk is
pre-packed to per-quad layout [q][p][dk][i][r] bf16; rk and w2^T are
bf16. The 2e-2 tolerance admits bf16 matmuls (~1e-3 rel err), which
halves DMA bytes and removes all device-side input transposes.

Per-core program:
  stage 1: for each quad of 4 pools: one DMA brings fkq [128, 4096];
           psum[t128, 512] accumulates xT.T @ fkq over 8 d-tiles;
           scalar_tensor_tensor applies the per-token routing weight
           w1[:, n], split across DVE (pools 0-1) and GpSimd (pools
           2-3) into two h accumulators; h = h_v + h_p; PE-transpose
           h -> hT bf16 [r, t].
  stage 2: g[n] = hT * broadcast(w2T[n]) in bf16, computed once and
           kept resident (16MB); rk slices stream as bf16; PSUM
           accumulates rk.T @ g over all 64 pools into 8 banks
           [d128, t512] per d-half; drained banks DMA to out [D, T]
           (host transposes back to [T, D]).
"""

from contextlib import ExitStack

import ml_dtypes
import numpy as np

import concourse.mybir as mybir
import concourse.tile as tile
from concourse import bacc
from concourse.bass_utils import run_bass_kernel_spmd
from concourse.masks import make_identity

F32 = mybir.dt.float32
BF16 = mybir.dt.bfloat16
NP_BF16 = ml_dtypes.bfloat16
MULT = mybir.AluOpType.mult
ADD = mybir.AluOpType.add

B, S, D, N, R = 4, 2048, 1024, 64, 128
N_CORES = 8
T = B * S // N_CORES  # tokens per core
TT = T // 128  # token tiles
DK = D // 128  # d tiles
NQ = N // 4  # stage-1 quads


def build_kernel(debug=False, repeat=1):
    """Build the per-core Bass program. repeat>1 runs the whole kernel
    body N times inside one program (device-timing aid only)."""
    nc = bacc.Bacc(None, target_bir_lowering=False, debug=debug)

    xT_d = nc.dram_tensor("xT", [D, T], BF16, kind="ExternalInput")
    w1_d = nc.dram_tensor("w1", [T, N], F32, kind="ExternalInput")
    w2T_d = nc.dram_tensor("w2T", [N, T], BF16, kind="ExternalInput")
    fk_d = nc.dram_tensor("fk", [NQ, 128, 4 * DK * 128], BF16, kind="ExternalInput")
    rk_d = nc.dram_tensor("rk", [N, R, D], BF16, kind="ExternalInput")
    out_d = nc.dram_tensor("out", [D, T], F32, kind="ExternalOutput")

    with tile.TileContext(nc) as tc, ExitStack() as ctx:
        sb_const = ctx.enter_context(tc.tile_pool(name="const", bufs=1))
        sb_xT = ctx.enter_context(tc.tile_pool(name="xT", bufs=DK))
        sb_w1 = ctx.enter_context(tc.tile_pool(name="w1p", bufs=TT))
        sb_fk = ctx.enter_context(tc.tile_pool(name="fkp", bufs=2))
        sb_h = ctx.enter_context(tc.tile_pool(name="hp", bufs=TT))
        sb_hT = ctx.enter_context(tc.tile_pool(name="hTp", bufs=1))
        sb_g = ctx.enter_context(tc.tile_pool(name="gp", bufs=N))
        sb_rk = ctx.enter_context(tc.tile_pool(name="rkp", bufs=4))
        sb_bc = ctx.enter_context(tc.tile_pool(name="bcp", bufs=3))
        sb_ot = ctx.enter_context(tc.tile_pool(name="otp", bufs=4))
        psum = ctx.enter_context(tc.tile_pool(name="ps", bufs=8, space="PSUM"))

        ident = sb_const.tile([128, 128], F32, tag="ident")
        make_identity(nc, ident[:])

        for _rep in range(repeat):
            _kernel_body(nc, tc, locals(), f"r{_rep}")

    nc.compile()
    return nc


def _kernel_body(nc, tc, env, pfx):
    sb_xT = env["sb_xT"]
    sb_w1 = env["sb_w1"]
    sb_fk = env["sb_fk"]
    sb_h = env["sb_h"]
    sb_hT = env["sb_hT"]
    sb_g = env["sb_g"]
    sb_rk = env["sb_rk"]
    sb_bc = env["sb_bc"]
    sb_ot = env["sb_ot"]
    psum = env["psum"]
    ident = env["ident"]
    xT_d = env["xT_d"]
    w1_d = env["w1_d"]
    w2T_d = env["w2T_d"]
    fk_d = env["fk_d"]
    rk_d = env["rk_d"]
    out_d = env["out_d"]

    if True:
        # ---- loads: xT tiles (pre-transposed on host), w1 tiles ----
        xT = [
            sb_xT.tile([128, T], BF16, tag="xT", name=f"{pfx}xT{i}")
            for i in range(DK)
        ]
        for dk in range(DK):
            nc.sync.dma_start(xT[dk][:], xT_d[dk * 128 : (dk + 1) * 128, :])
        w1 = []
        for tt in range(TT):
            t1 = sb_w1.tile([128, N], F32, tag="w1")
            nc.sync.dma_start(t1[:], w1_d[tt * 128 : (tt + 1) * 128, :])
            w1.append(t1)

        # ---- stage 1: h[t, r] accumulation over all pools ----
        h_v = [
            sb_h.tile([128, R], F32, tag="h", name=f"{pfx}hv{i}") for i in range(TT)
        ]
        for tt in range(TT):
            nc.vector.memset(h_v[tt][:], 0.0)

        for q in range(NQ):
            fkq = sb_fk.tile([128, 4 * DK * 128], BF16, tag="fk")
            nc.sync.dma_start(fkq[:], fk_d[q, :, :])
            for ttg in range(TT // 4):
                tts = range(ttg * 4, ttg * 4 + 4)
                hps = {
                    tt: psum.tile([128, 512], F32, tag="ps", name=f"{pfx}hps{tt}")
                    for tt in tts
                }
                for dk in range(DK):
                    for tt in tts:
                        nc.tensor.matmul(
                            hps[tt][:],
                            xT[dk][:, tt * 128 : (tt + 1) * 128],
                            fkq[:, dk * 512 : (dk + 1) * 512],
                            start=(dk == 0),
                            stop=(dk == DK - 1),
                        )
                for tt in tts:
                    for i in range(4):
                        n = q * 4 + i
                        nc.vector.scalar_tensor_tensor(
                            h_v[tt][:],
                            hps[tt][:, i * 128 : (i + 1) * 128],
                            w1[tt][:, n : n + 1],
                            h_v[tt][:],
                            MULT,
                            ADD,
                        )

        # ---- hT (bf16) ----
        hT = sb_hT.tile([128, T], BF16, tag="hT")
        for tt in range(TT):
            tp = psum.tile([128, 128], F32, tag="ps")
            nc.tensor.transpose(tp[:], h_v[tt][:], ident[:])
            nc.vector.tensor_copy(hT[:, tt * 128 : (tt + 1) * 128], tp[:])

        # ---- stage 2: out[d, t] accumulation over all pools ----
        g = [
            sb_g.tile([128, T], BF16, tag="g", name=f"{pfx}g{i}") for i in range(N)
        ]
        dk_half = DK // 2
        for ph in range(2):
            ops = [
                psum.tile([128, 512], F32, tag="ps", name=f"{pfx}ops{ph}_{i}")
                for i in range(DK)
            ]
            for n in range(N):
                if ph == 0:
                    bc = sb_bc.tile([128, T], BF16, tag="bc")
                    nc.scalar.dma_start(
                        bc[:], w2T_d[n : n + 1, :].partition_broadcast(128)
                    )
                    nc.vector.tensor_mul(g[n][:], hT[:], bc[:])
                rkh = sb_rk.tile([128, 512], BF16, tag="rk")
                nc.sync.dma_start(
                    rkh[:], rk_d[n, :, ph * 512 : (ph + 1) * 512]
                )
                for dki in range(dk_half):
                    for t5 in range(2):
                        nc.tensor.matmul(
                            ops[dki * 2 + t5][:],
                            rkh[:, dki * 128 : (dki + 1) * 128],
                            g[n][:, t5 * 512 : (t5 + 1) * 512],
                            start=(n == 0),
                            stop=(n == N - 1),
                        )
            for dki in range(dk_half):
                for t5 in range(2):
                    ot = sb_ot.tile([128, 512], F32, tag="ot")
                    nc.vector.tensor_copy(ot[:], ops[dki * 2 + t5][:])
                    nc.sync.dma_start(
                        out_d[
                            (ph * dk_half + dki) * 128 : (ph * dk_half + dki + 1) * 128,
                            t5 * 512 : (t5 + 1) * 512,
                        ],
                        ot[:],
                    )


_NC_CACHE = {}


def _get_nc():
    if "nc" not in _NC_CACHE:
        _NC_CACHE["nc"] = build_kernel(debug=False)
    return _NC_CACHE["nc"]


def _shard_inputs(x, feature_know_w, restore_know_w, feature_know, restore_know):
    xc = np.asarray(x, dtype=np.float32).reshape(N_CORES, T, D)
    xT = xc.transpose(0, 2, 1).astype(NP_BF16)  # [C, D, T]
    w1 = np.ascontiguousarray(
        np.asarray(feature_know_w, dtype=np.float32).reshape(N_CORES, T, N)
    )
    w2T = (
        np.asarray(restore_know_w, dtype=np.float32)
        .reshape(N_CORES, T, N)
        .transpose(0, 2, 1)
        .astype(NP_BF16)
    )  # [C, N, T]
    fk = (
        np.asarray(feature_know, dtype=np.float32)
        .reshape(NQ, 4, DK, 128, R)
        .transpose(0, 3, 2, 1, 4)  # [q, p, dk, i, r]
        .astype(NP_BF16)
        .reshape(NQ, 128, 4 * DK * 128)
    )
    rk = np.asarray(restore_know, dtype=np.float32).astype(NP_BF16)  # [N, R, D]
    in_maps = []
    for c in range(N_CORES):
        in_maps.append(
            {
                "xT": np.ascontiguousarray(xT[c]),
                "w1": w1[c],
                "w2T": np.ascontiguousarray(w2T[c]),
                "fk": fk,
                "rk": rk,
            }
        )
    return in_maps


def _unshard_out(per_core_outs):
    """per_core_outs: list of [D, T] arrays -> [B, S, D]."""
    stacked = np.stack(per_core_outs, axis=0)  # [C, D, T]
    return np.ascontiguousarray(stacked.transpose(0, 2, 1)).reshape(B, S, D)


def run(in_maps, **kwargs):
    nc = _get_nc()
    return run_bass_kernel_spmd(nc, in_maps, core_ids=list(range(N_CORES)), **kwargs)


def kernel(x, feature_know_w, restore_know_w, feature_know, restore_know, **_):
    in_maps = _shard_inputs(
        x, feature_know_w, restore_know_w, feature_know, restore_know
    )
    res = run(in_maps)
    return _unshard_out([r["out"] for r in res.results])


# revision 37
# speedup vs baseline: 7.2277x; 7.2277x over previous
"""Trainium2 Bass kernel for nn_KnowledgeCircuit (moe_routing).

  h   = einsum('bsd,ndr,bsn->bsr', x, feature_know, feature_know_w)
  out = einsum('bsr,bsn,nrd->bsd', h, restore_know_w, restore_know)

Shapes: B=4, S=2048, D=1024, N=64, R=128.

Sharding: data-parallel over the B*S = 8192 tokens -> 1024 tokens per
NeuronCore across 8 cores; neuron pools (fk, rk) replicated. No
collectives.

Host prep: x is pre-transposed to [D, T] and packed to bf16; fk is
pre-packed to per-quad layout [q][p][dk][i][r] bf16; rk and w2^T are
bf16. The 2e-2 tolerance admits bf16 matmuls (~1e-3 rel err), which
halves DMA bytes and removes all device-side input transposes.

Per-core program:
  stage 1: for each quad of 4 pools: one DMA brings fkq [128, 4096];
           psum[t128, 512] accumulates xT.T @ fkq over 8 d-tiles;
           scalar_tensor_tensor applies the per-token routing weight
           w1[:, n], split across DVE (pools 0-1) and GpSimd (pools
           2-3) into two h accumulators; h = h_v + h_p; PE-transpose
           h -> hT bf16 [r, t].
  stage 2: g[n] = hT * broadcast(w2T[n]) in bf16, computed once and
           kept resident (16MB); rk slices stream as bf16; PSUM
           accumulates rk.T @ g over all 64 pools into 8 banks
           [d128, t512] per d-half; drained banks DMA to out [D, T]
           (host transposes back to [T, D]).
"""

from contextlib import ExitStack

import ml_dtypes
import numpy as np

import concourse.mybir as mybir
import concourse.tile as tile
from concourse import bacc
from concourse.bass_utils import run_bass_kernel_spmd
from concourse.masks import make_identity

F32 = mybir.dt.float32
BF16 = mybir.dt.bfloat16
NP_BF16 = ml_dtypes.bfloat16
MULT = mybir.AluOpType.mult
ADD = mybir.AluOpType.add

B, S, D, N, R = 4, 2048, 1024, 64, 128
N_CORES = 8
T = B * S // N_CORES  # tokens per core
TT = T // 128  # token tiles
DK = D // 128  # d tiles
NQ = N // 4  # stage-1 quads


def build_kernel(debug=False, repeat=1):
    """Build the per-core Bass program. repeat>1 runs the whole kernel
    body N times inside one program (device-timing aid only)."""
    nc = bacc.Bacc(None, target_bir_lowering=False, debug=debug)

    xT_d = nc.dram_tensor("xT", [D, T], BF16, kind="ExternalInput")
    w1_d = nc.dram_tensor("w1", [T, N], F32, kind="ExternalInput")
    w2T_d = nc.dram_tensor("w2T", [N, T], BF16, kind="ExternalInput")
    fk_d = nc.dram_tensor("fk", [NQ, 128, 4 * DK * 128], BF16, kind="ExternalInput")
    rk_d = nc.dram_tensor("rk", [N, R, D], BF16, kind="ExternalInput")
    out_d = nc.dram_tensor("out", [D, T], F32, kind="ExternalOutput")

    with tile.TileContext(nc) as tc, ExitStack() as ctx:
        sb_const = ctx.enter_context(tc.tile_pool(name="const", bufs=1))
        sb_xT = ctx.enter_context(tc.tile_pool(name="xT", bufs=DK))
        sb_w1 = ctx.enter_context(tc.tile_pool(name="w1p", bufs=TT))
        sb_fk = ctx.enter_context(tc.tile_pool(name="fkp", bufs=2))
        sb_h = ctx.enter_context(tc.tile_pool(name="hp", bufs=TT))
        sb_hT = ctx.enter_context(tc.tile_pool(name="hTp", bufs=1))
        sb_g = ctx.enter_context(tc.tile_pool(name="gp", bufs=N))
        sb_rk = ctx.enter_context(tc.tile_pool(name="rkp", bufs=4))
        sb_bc = ctx.enter_context(tc.tile_pool(name="bcp", bufs=8))
        sb_ot = ctx.enter_context(tc.tile_pool(name="otp", bufs=4))
        psum = ctx.enter_context(tc.tile_pool(name="ps", bufs=8, space="PSUM"))

        ident = sb_const.tile([128, 128], F32, tag="ident")
        make_identity(nc, ident[:])

        for _rep in range(repeat):
            _kernel_body(nc, tc, locals(), f"r{_rep}")

    nc.compile()
    return nc


def _kernel_body(nc, tc, env, pfx):
    sb_xT = env["sb_xT"]
    sb_w1 = env["sb_w1"]
    sb_fk = env["sb_fk"]
    sb_h = env["sb_h"]
    sb_hT = env["sb_hT"]
    sb_g = env["sb_g"]
    sb_rk = env["sb_rk"]
    sb_bc = env["sb_bc"]
    sb_ot = env["sb_ot"]
    psum = env["psum"]
    ident = env["ident"]
    xT_d = env["xT_d"]
    w1_d = env["w1_d"]
    w2T_d = env["w2T_d"]
    fk_d = env["fk_d"]
    rk_d = env["rk_d"]
    out_d = env["out_d"]

    if True:
        # ---- loads: xT tiles (pre-transposed on host) on Act queue so the
        # first fkq DMA (SP) runs in parallel; w1 after fkq[0] on SP ----
        xT = [
            sb_xT.tile([128, T], BF16, tag="xT", name=f"{pfx}xT{i}")
            for i in range(DK)
        ]
        # xT[0] on SP ahead of fkq so PE's first Ldweights starts early
        # (Act's queue opens with a ~1.3us LoadActFuncSet); rest on Act
        nc.sync.dma_start(xT[0][:, :128], xT_d[0:128, :128])
        nc.sync.dma_start(xT[0][:, 128:], xT_d[0:128, 128:])
        for dk in range(1, DK):
            nc.scalar.dma_start(xT[dk][:], xT_d[dk * 128 : (dk + 1) * 128, :])

        # ---- stage 1: h[t, r] accumulation over all pools ----
        h_v = [
            sb_h.tile([128, R], F32, tag="h", name=f"{pfx}hv{i}") for i in range(TT)
        ]
        for tt in range(TT):
            nc.vector.memset(h_v[tt][:], 0.0)

        hT = sb_hT.tile([128, T], BF16, tag="hT")

        def emit_hT(tt):
            # transpose h_v[tt] -> hT column block, copy split DVE/Act
            tp = psum.tile([128, 128], F32, tag="ps")
            nc.tensor.transpose(tp[:], h_v[tt][:], ident[:])
            dst = hT[:, tt * 128 : (tt + 1) * 128]
            if tt % 2 == 0:
                nc.vector.tensor_copy(dst, tp[:])
            else:
                nc.scalar.activation(dst, tp[:], mybir.ActivationFunctionType.Copy)

        w1 = []
        for q in range(NQ):
            last_q = q == NQ - 1
            fkq = sb_fk.tile([128, 4 * DK * 128], BF16, tag="fk")
            if q == 0:
                # split the first fkq DMA so dk-0 matmuls start sooner
                quarter = 4 * DK * 128 // 4
                for j in range(4):
                    nc.sync.dma_start(
                        fkq[:, j * quarter : (j + 1) * quarter],
                        fk_d[q, :, j * quarter : (j + 1) * quarter],
                    )
                # w1 tiles: needed only by the stt drain, load after fkq[0]
                for tt in range(TT):
                    t1 = sb_w1.tile([128, N], F32, tag="w1")
                    nc.sync.dma_start(t1[:], w1_d[tt * 128 : (tt + 1) * 128, :])
                    w1.append(t1)
            else:
                nc.sync.dma_start(fkq[:], fk_d[q, :, :])
            groups = [range(0, 4), range(4, 8)]
            if last_q:
                # shrinking tail groups + early hT emission: transposes for
                # already-drained token tiles run between the remaining
                # matmul groups, so only tt7's chain gates stage 2
                groups = [range(0, 4), range(4, 6), range(6, 7), range(7, 8)]
            hT_emitted = 0
            for gi, tts in enumerate(groups):
                if last_q and gi >= 2:
                    # tiles of groups < gi-1 have fully drained by now
                    ready = groups[gi - 2].stop if gi >= 2 else 0
                    while hT_emitted < ready:
                        emit_hT(hT_emitted)
                        hT_emitted += 1
                hps = {
                    tt: psum.tile([128, 512], F32, tag="ps", name=f"{pfx}hps{tt}")
                    for tt in tts
                }
                for dk in range(DK):
                    for tt in tts:
                        nc.tensor.matmul(
                            hps[tt][:],
                            xT[dk][:, tt * 128 : (tt + 1) * 128],
                            fkq[:, dk * 512 : (dk + 1) * 512],
                            start=(dk == 0),
                            stop=(dk == DK - 1),
                        )
                for tt in tts:
                    for i in range(4):
                        n = q * 4 + i
                        nc.vector.scalar_tensor_tensor(
                            h_v[tt][:],
                            hps[tt][:, i * 128 : (i + 1) * 128],
                            w1[tt][:, n : n + 1],
                            h_v[tt][:],
                            MULT,
                            ADD,
                        )

        while hT_emitted < TT:
            emit_hT(hT_emitted)
            hT_emitted += 1

        # ---- stage 2: out[d, t] accumulation over all pools, 4 d-quarter
        # passes of 4 PSUM banks each (pass k+1 accumulates while pass k
        # drains; rkh stays exclusive on the SP queue) ----
        g = [
            sb_g.tile([128, T], BF16, tag="g", name=f"{pfx}g{i}") for i in range(N)
        ]
        passes = [(0, 2), (2, 4), (4, 6), (6, 7), (7, 8)]  # d-block ranges
        for pq, (d0, d1) in enumerate(passes):
            ndk = d1 - d0
            last_pass = pq == len(passes) - 1
            ops = [
                psum.tile([128, 512], F32, tag="ps", name=f"{pfx}ops{pq}_{i}")
                for i in range(2 * ndk)
            ]
            for n in range(N):
                if pq == 0:
                    bc = sb_bc.tile([128, T], BF16, tag="bc")
                    nc.scalar.dma_start(
                        bc[:], w2T_d[n : n + 1, :].partition_broadcast(128)
                    )
                    nc.vector.tensor_mul(g[n][:], hT[:], bc[:])
                if ndk == 1:
                    if n % 2 == 0:
                        rkh2 = sb_rk.tile([128, 256], BF16, tag="rk")
                        nc.sync.dma_start(
                            rkh2[:], rk_d[n : n + 2, :, d0 * 128 : d1 * 128].rearrange("a b c -> b a c")
                        )
                    rkh = rkh2[:, (n % 2) * 128 : (n % 2) * 128 + 128]
                else:
                    rkh_t = sb_rk.tile([128, 128 * ndk], BF16, tag="rk")
                    nc.sync.dma_start(
                        rkh_t[:], rk_d[n, :, d0 * 128 : d1 * 128]
                    )
                    rkh = rkh_t[:]
                for dki in range(ndk):
                    for t5 in range(2):
                        nc.tensor.matmul(
                            ops[dki * 2 + t5][:],
                            rkh[:, dki * 128 : (dki + 1) * 128] if ndk > 1 else rkh,
                            g[n][:, t5 * 512 : (t5 + 1) * 512],
                            start=(n == 0),
                            stop=(n == N - 1),
                        )
            for dki in range(ndk):
                for t5 in range(2):
                    i = dki * 2 + t5
                    ot = sb_ot.tile([128, 512], F32, tag="ot")
                    if i % 2 == 0:
                        nc.vector.tensor_copy(ot[:], ops[i][:])
                    else:
                        nc.scalar.activation(
                            ot[:], ops[i][:], mybir.ActivationFunctionType.Copy
                        )
                    # during earlier passes SP is saturated by rkh; on the
                    # final pass it is idle, so split drain DMAs across queues
                    dma_eng = nc.sync if (last_pass and i % 2 == 0) else nc.scalar
                    dma_eng.dma_start(
                        out_d[
                            (d0 + dki) * 128 : (d0 + dki + 1) * 128,
                            t5 * 512 : (t5 + 1) * 512,
                        ],
                        ot[:],
                    )


_NC_CACHE = {}


def _get_nc():
    if "nc" not in _NC_CACHE:
        _NC_CACHE["nc"] = build_kernel(debug=False)
    return _NC_CACHE["nc"]


def _shard_inputs(x, feature_know_w, restore_know_w, feature_know, restore_know):
    xc = np.asarray(x, dtype=np.float32).reshape(N_CORES, T, D)
    xT = xc.transpose(0, 2, 1).astype(NP_BF16)  # [C, D, T]
    w1 = np.ascontiguousarray(
        np.asarray(feature_know_w, dtype=np.float32).reshape(N_CORES, T, N)
    )
    w2T = (
        np.asarray(restore_know_w, dtype=np.float32)
        .reshape(N_CORES, T, N)
        .transpose(0, 2, 1)
        .astype(NP_BF16)
    )  # [C, N, T]
    fk = (
        np.asarray(feature_know, dtype=np.float32)
        .reshape(NQ, 4, DK, 128, R)
        .transpose(0, 3, 2, 1, 4)  # [q, p, dk, i, r]
        .astype(NP_BF16)
        .reshape(NQ, 128, 4 * DK * 128)
    )
    rk = np.asarray(restore_know, dtype=np.float32).astype(NP_BF16)  # [N, R, D]
    in_maps = []
    for c in range(N_CORES):
        in_maps.append(
            {
                "xT": np.ascontiguousarray(xT[c]),
                "w1": w1[c],
                "w2T": np.ascontiguousarray(w2T[c]),
                "fk": fk,
                "rk": rk,
            }
        )
    return in_maps


def _unshard_out(per_core_outs):
    """per_core_outs: list of [D, T] arrays -> [B, S, D]."""
    stacked = np.stack(per_core_outs, axis=0)  # [C, D, T]
    return np.ascontiguousarray(stacked.transpose(0, 2, 1)).reshape(B, S, D)


def run(in_maps, **kwargs):
    nc = _get_nc()
    return run_bass_kernel_spmd(nc, in_maps, core_ids=list(range(N_CORES)), **kwargs)


def kernel(x, feature_know_w, restore_know_w, feature_know, restore_know, **_):
    in_maps = _shard_inputs(
        x, feature_know_w, restore_know_w, feature_know, restore_know
    )
    res = run(in_maps)
    return _unshard_out([r["out"] for r in res.results])


# revision 41
# speedup vs baseline: 7.2450x; 1.0024x over previous
"""Trainium2 Bass kernel for nn_KnowledgeCircuit (moe_routing).

  h   = einsum('bsd,ndr,bsn->bsr', x, feature_know, feature_know_w)
  out = einsum('bsr,bsn,nrd->bsd', h, restore_know_w, restore_know)

Shapes: B=4, S=2048, D=1024, N=64, R=128.

Sharding: data-parallel over the B*S = 8192 tokens -> 1024 tokens per
NeuronCore across 8 cores; neuron pools (fk, rk) replicated. No
collectives.

Host prep: x is pre-transposed to [D, T] and packed to bf16; fk is
pre-packed to per-quad layout [q][p][dk][i][r] bf16; rk and w2^T are
bf16. The 2e-2 tolerance admits bf16 matmuls (~1e-3 rel err), which
halves DMA bytes and removes all device-side input transposes.

Per-core program:
  stage 1: for each quad of 4 pools: one DMA brings fkq [128, 4096];
           psum[t128, 512] accumulates xT.T @ fkq over 8 d-tiles;
           scalar_tensor_tensor applies the per-token routing weight
           w1[:, n], split across DVE (pools 0-1) and GpSimd (pools
           2-3) into two h accumulators; h = h_v + h_p; PE-transpose
           h -> hT bf16 [r, t].
  stage 2: g[n] = hT * broadcast(w2T[n]) in bf16, computed once and
           kept resident (16MB); rk slices stream as bf16; PSUM
           accumulates rk.T @ g over all 64 pools into 8 banks
           [d128, t512] per d-half; drained banks DMA to out [D, T]
           (host transposes back to [T, D]).
"""

from contextlib import ExitStack

import ml_dtypes
import numpy as np

import concourse.mybir as mybir
import concourse.tile as tile
from concourse import bacc
from concourse.bass_utils import run_bass_kernel_spmd
from concourse.masks import make_identity

F32 = mybir.dt.float32
BF16 = mybir.dt.bfloat16
NP_BF16 = ml_dtypes.bfloat16
MULT = mybir.AluOpType.mult
ADD = mybir.AluOpType.add

B, S, D, N, R = 4, 2048, 1024, 64, 128
N_CORES = 8
T = B * S // N_CORES  # tokens per core
TT = T // 128  # token tiles
DK = D // 128  # d tiles
NQ = N // 4  # stage-1 quads


def build_kernel(debug=False, repeat=1):
    """Build the per-core Bass program. repeat>1 runs the whole kernel
    body N times inside one program (device-timing aid only)."""
    nc = bacc.Bacc(None, target_bir_lowering=False, debug=debug)

    xT_d = nc.dram_tensor("xT", [D, T], BF16, kind="ExternalInput")
    w1_d = nc.dram_tensor("w1", [T, N], F32, kind="ExternalInput")
    w2T_d = nc.dram_tensor("w2T", [N, T], BF16, kind="ExternalInput")
    fk_d = nc.dram_tensor("fk", [NQ, 128, 4 * DK * 128], BF16, kind="ExternalInput")
    rk_d = nc.dram_tensor("rk", [N, R, D], BF16, kind="ExternalInput")
    out_d = nc.dram_tensor("out", [D, T], F32, kind="ExternalOutput")

    with tile.TileContext(nc) as tc, ExitStack() as ctx:
        sb_const = ctx.enter_context(tc.tile_pool(name="const", bufs=1))
        sb_xT = ctx.enter_context(tc.tile_pool(name="xT", bufs=DK))
        sb_w1 = ctx.enter_context(tc.tile_pool(name="w1p", bufs=TT))
        sb_fk = ctx.enter_context(tc.tile_pool(name="fkp", bufs=2))
        sb_h = ctx.enter_context(tc.tile_pool(name="hp", bufs=TT))
        sb_hT = ctx.enter_context(tc.tile_pool(name="hTp", bufs=1))
        sb_g = ctx.enter_context(tc.tile_pool(name="gp", bufs=N))
        sb_rk = ctx.enter_context(tc.tile_pool(name="rkp", bufs=9))
        sb_bc = ctx.enter_context(tc.tile_pool(name="bcp", bufs=8))
        sb_ot = ctx.enter_context(tc.tile_pool(name="otp", bufs=4))
        psum = ctx.enter_context(tc.tile_pool(name="ps", bufs=8, space="PSUM"))

        ident = sb_const.tile([128, 128], F32, tag="ident")
        make_identity(nc, ident[:])

        for _rep in range(repeat):
            _kernel_body(nc, tc, locals(), f"r{_rep}")

    nc.compile()
    return nc


def _kernel_body(nc, tc, env, pfx):
    sb_xT = env["sb_xT"]
    sb_w1 = env["sb_w1"]
    sb_fk = env["sb_fk"]
    sb_h = env["sb_h"]
    sb_hT = env["sb_hT"]
    sb_g = env["sb_g"]
    sb_rk = env["sb_rk"]
    sb_bc = env["sb_bc"]
    sb_ot = env["sb_ot"]
    psum = env["psum"]
    ident = env["ident"]
    xT_d = env["xT_d"]
    w1_d = env["w1_d"]
    w2T_d = env["w2T_d"]
    fk_d = env["fk_d"]
    rk_d = env["rk_d"]
    out_d = env["out_d"]

    if True:
        # ---- loads: xT tiles (pre-transposed on host) on Act queue so the
        # first fkq DMA (SP) runs in parallel; w1 after fkq[0] on SP ----
        xT = [
            sb_xT.tile([128, T], BF16, tag="xT", name=f"{pfx}xT{i}")
            for i in range(DK)
        ]
        # xT[0] on SP ahead of fkq so PE's first Ldweights starts early
        # (Act's queue opens with a ~1.3us LoadActFuncSet); rest on Act
        nc.sync.dma_start(xT[0][:, :128], xT_d[0:128, :128])
        nc.sync.dma_start(xT[0][:, 128:], xT_d[0:128, 128:])
        for dk in range(1, DK):
            nc.scalar.dma_start(xT[dk][:], xT_d[dk * 128 : (dk + 1) * 128, :])

        # ---- stage 1: h[t, r] accumulation over all pools ----
        h_v = [
            sb_h.tile([128, R], F32, tag="h", name=f"{pfx}hv{i}") for i in range(TT)
        ]
        for tt in range(TT):
            nc.vector.memset(h_v[tt][:], 0.0)

        hT = sb_hT.tile([128, T], BF16, tag="hT")

        def emit_hT(tt):
            # transpose h_v[tt] -> hT column block, copy split DVE/Act
            tp = psum.tile([128, 128], F32, tag="ps")
            nc.tensor.transpose(tp[:], h_v[tt][:], ident[:])
            dst = hT[:, tt * 128 : (tt + 1) * 128]
            if tt % 2 == 0:
                nc.vector.tensor_copy(dst, tp[:])
            else:
                nc.scalar.activation(dst, tp[:], mybir.ActivationFunctionType.Copy)

        # stage-2 g tiles and warm-window state (referenced from the tail of
        # the stage-1 loop)
        g = [
            sb_g.tile([128, T], BF16, tag="g", name=f"{pfx}g{i}") for i in range(N)
        ]
        WARM = 6
        bc_warm = []

        w1 = []
        for q in range(NQ):
            last_q = q == NQ - 1
            fkq = sb_fk.tile([128, 4 * DK * 128], BF16, tag="fk")
            if q == 0:
                # split the first fkq DMA so dk-0 matmuls start sooner
                quarter = 4 * DK * 128 // 4
                for j in range(4):
                    nc.sync.dma_start(
                        fkq[:, j * quarter : (j + 1) * quarter],
                        fk_d[q, :, j * quarter : (j + 1) * quarter],
                    )
                # w1 tiles: needed only by the stt drain, load after fkq[0]
                for tt in range(TT):
                    t1 = sb_w1.tile([128, N], F32, tag="w1")
                    nc.sync.dma_start(t1[:], w1_d[tt * 128 : (tt + 1) * 128, :])
                    w1.append(t1)
            else:
                nc.sync.dma_start(fkq[:], fk_d[q, :, :])
            groups = [range(0, 4), range(4, 8)]
            if last_q:
                # shrinking tail groups + early hT emission: transposes for
                # already-drained token tiles run between the remaining
                # matmul groups, so only tt7's chain gates stage 2
                groups = [range(0, 4), range(4, 6), range(6, 7), range(7, 8)]
            hT_emitted = 0
            for gi, tts in enumerate(groups):
                if last_q and gi >= 2:
                    # tiles of groups < gi-1 have fully drained by now
                    ready = groups[gi - 2].stop if gi >= 2 else 0
                    while hT_emitted < ready:
                        emit_hT(hT_emitted)
                        hT_emitted += 1
                if last_q and gi == 3:
                    # warm-window prep: bc for the first WARM pools (on SP,
                    # which is idle here) and the lower-half g muls, which
                    # depend only on hT columns of tts 0-3 (already copied)
                    for n in range(WARM):
                        bcw = sb_bc.tile([128, T], BF16, tag="bc")
                        nc.sync.dma_start(
                            bcw[:], w2T_d[n : n + 1, :].partition_broadcast(128)
                        )
                        bc_warm.append(bcw)
                    for n in range(WARM):
                        nc.vector.tensor_mul(
                            g[n][:, : T // 2],
                            hT[:, : T // 2],
                            bc_warm[n][:, : T // 2],
                        )
                hps = {
                    tt: psum.tile([128, 512], F32, tag="ps", name=f"{pfx}hps{tt}")
                    for tt in tts
                }
                for dk in range(DK):
                    for tt in tts:
                        nc.tensor.matmul(
                            hps[tt][:],
                            xT[dk][:, tt * 128 : (tt + 1) * 128],
                            fkq[:, dk * 512 : (dk + 1) * 512],
                            start=(dk == 0),
                            stop=(dk == DK - 1),
                        )
                for tt in tts:
                    for i in range(4):
                        n = q * 4 + i
                        nc.vector.scalar_tensor_tensor(
                            h_v[tt][:],
                            hps[tt][:, i * 128 : (i + 1) * 128],
                            w1[tt][:, n : n + 1],
                            h_v[tt][:],
                            MULT,
                            ADD,
                        )

        while hT_emitted < TT - 2:
            emit_hT(hT_emitted)
            hT_emitted += 1

        # ---- stage 2: out[d, t] accumulation over all pools, d-block
        # passes (pass k+1 accumulates while pass k drains; rkh stays
        # mostly on the SP queue).
        # Pass 0 opens with a warm window: t5=0 matmuls for the first WARM
        # pools depend only on the lower hT half (ready before the last
        # stage-1 matmul), hiding the tt6/tt7 transpose chain. ----
        passes = [(0, 2), (2, 4), (4, 6), (6, 7), (7, 8)]  # d-block ranges
        for pq, (d0, d1) in enumerate(passes):
            ndk = d1 - d0
            last_pass = pq == len(passes) - 1
            ops = [
                psum.tile([128, 512], F32, tag="ps", name=f"{pfx}ops{pq}_{i}")
                for i in range(2 * ndk)
            ]
            if pq == 0:
                # warm window: t5=0 matmuls for pools 0..WARM-1
                warm_rkh = []
                for n in range(WARM):
                    rkh_t = sb_rk.tile([128, 256], BF16, tag="rk")
                    nc.sync.dma_start(rkh_t[:], rk_d[n, :, 0:256])
                    warm_rkh.append(rkh_t)
                    for dki in range(2):
                        nc.tensor.matmul(
                            ops[dki * 2][:],
                            rkh_t[:, dki * 128 : (dki + 1) * 128],
                            g[n][:, 0:512],
                            start=(n == 0),
                            stop=False,
                        )
                    if n == 2:
                        # tt6/tt7 transposes: their stt chains finished
                        # during the warm matmuls above
                        emit_hT(TT - 2)
                        emit_hT(TT - 1)
                # upper-half g for warm pools (needs the tt4-7 hT copies)
                for n in range(WARM):
                    nc.vector.tensor_mul(
                        g[n][:, T // 2 :],
                        hT[:, T // 2 :],
                        bc_warm[n][:, T // 2 :],
                    )
                # t5=1 backfill for warm pools
                for n in range(WARM):
                    for dki in range(2):
                        nc.tensor.matmul(
                            ops[dki * 2 + 1][:],
                            warm_rkh[n][:, dki * 128 : (dki + 1) * 128],
                            g[n][:, 512:1024],
                            start=(n == 0),
                            stop=False,
                        )
            for n in range(WARM if pq == 0 else 0, N):
                if pq == 0:
                    bc = sb_bc.tile([128, T], BF16, tag="bc")
                    nc.scalar.dma_start(
                        bc[:], w2T_d[n : n + 1, :].partition_broadcast(128)
                    )
                    nc.vector.tensor_mul(g[n][:], hT[:], bc[:])
                if ndk == 1:
                    if n % 2 == 0:
                        rkh2 = sb_rk.tile([128, 256], BF16, tag="rk")
                        nc.sync.dma_start(
                            rkh2[:], rk_d[n : n + 2, :, d0 * 128 : d1 * 128].rearrange("a b c -> b a c")
                        )
                    rkh = rkh2[:, (n % 2) * 128 : (n % 2) * 128 + 128]
                else:
                    rkh_t = sb_rk.tile([128, 128 * ndk], BF16, tag="rk")
                    nc.sync.dma_start(
                        rkh_t[:], rk_d[n, :, d0 * 128 : d1 * 128]
                    )
                    rkh = rkh_t[:]
                for dki in range(ndk):
                    for t5 in range(2):
                        nc.tensor.matmul(
                            ops[dki * 2 + t5][:],
                            rkh[:, dki * 128 : (dki + 1) * 128] if ndk > 1 else rkh,
                            g[n][:, t5 * 512 : (t5 + 1) * 512],
                            start=(n == 0),
                            stop=(n == N - 1),
                        )
            for dki in range(ndk):
                for t5 in range(2):
                    i = dki * 2 + t5
                    ot = sb_ot.tile([128, 512], F32, tag="ot")
                    if i % 2 == 0:
                        nc.vector.tensor_copy(ot[:], ops[i][:])
                    else:
                        nc.scalar.activation(
                            ot[:], ops[i][:], mybir.ActivationFunctionType.Copy
                        )
                    # during earlier passes SP is saturated by rkh; on the
                    # final pass it is idle, so split drain DMAs across queues
                    dma_eng = nc.sync if (last_pass and i % 2 == 0) else nc.scalar
                    dma_eng.dma_start(
                        out_d[
                            (d0 + dki) * 128 : (d0 + dki + 1) * 128,
                            t5 * 512 : (t5 + 1) * 512,
                        ],
                        ot[:],
                    )


_NC_CACHE = {}


def _get_nc():
    if "nc" not in _NC_CACHE:
        _NC_CACHE["nc"] = build_kernel(debug=False)
    return _NC_CACHE["nc"]


def _shard_inputs(x, feature_know_w, restore_know_w, feature_know, restore_know):
    xc = np.asarray(x, dtype=np.float32).reshape(N_CORES, T, D)
    xT = xc.transpose(0, 2, 1).astype(NP_BF16)  # [C, D, T]
    w1 = np.ascontiguousarray(
        np.asarray(feature_know_w, dtype=np.float32).reshape(N_CORES, T, N)
    )
    w2T = (
        np.asarray(restore_know_w, dtype=np.float32)
        .reshape(N_CORES, T, N)
        .transpose(0, 2, 1)
        .astype(NP_BF16)
    )  # [C, N, T]
    fk = (
        np.asarray(feature_know, dtype=np.float32)
        .reshape(NQ, 4, DK, 128, R)
        .transpose(0, 3, 2, 1, 4)  # [q, p, dk, i, r]
        .astype(NP_BF16)
        .reshape(NQ, 128, 4 * DK * 128)
    )
    rk = np.asarray(restore_know, dtype=np.float32).astype(NP_BF16)  # [N, R, D]
    in_maps = []
    for c in range(N_CORES):
        in_maps.append(
            {
                "xT": np.ascontiguousarray(xT[c]),
                "w1": w1[c],
                "w2T": np.ascontiguousarray(w2T[c]),
                "fk": fk,
                "rk": rk,
            }
        )
    return in_maps


def _unshard_out(per_core_outs):
    """per_core_outs: list of [D, T] arrays -> [B, S, D]."""
    stacked = np.stack(per_core_outs, axis=0)  # [C, D, T]
    return np.ascontiguousarray(stacked.transpose(0, 2, 1)).reshape(B, S, D)


def run(in_maps, **kwargs):
    nc = _get_nc()
    return run_bass_kernel_spmd(nc, in_maps, core_ids=list(range(N_CORES)), **kwargs)


def kernel(x, feature_know_w, restore_know_w, feature_know, restore_know, **_):
    in_maps = _shard_inputs(
        x, feature_know_w, restore_know_w, feature_know, restore_know
    )
    res = run(in_maps)
    return _unshard_out([r["out"] for r in res.results])
